# revision 2
# baseline (speedup 1.0000x reference)
"""ClosestPool1D TRN2 kernel: for src/tgt, find the 2nd-nearest neighbor of
each shortcut coord among 32768 coords (3D, squared L2) and gather its
feature row.

Per core (M sharded 8 ways -> 1024 queries/side/core), byte-exact with the
neuron-backend reference:
  - PE: Z = 2*a@b (K=3 fp32, reproduces jit_matmul bits).  The 4 512-col
    chunks of each 2048-span run CONCURRENTLY in 4 PE row groups via
    tile_position (bit-identical, ~3x faster than sequential fp32 matmuls).
  - ACT: drains each PSUM span to SBUF (pure copy).
  - GPSIMD (or DVE, per ADD knob): score = (negb2 + (-a2)) + Z via
    scalar_tensor_tensor -- reproduces the reference's fl(a2+b2) then
    single-subtract rounding exactly.
  - DVE: Max8 + MaxIndex per 8192-col SBUF quarter-row (the irreducible
    2-pass selection), stable merge of 8 candidate slots, one-hot dot for
    the 2nd-best global index; indirect DMA gathers the feature row.
"""
import numpy as np

import concourse.bass as bass
import concourse.mybir as mybir
from concourse.tile import TileContext
from concourse.bass_utils import run_bass_kernel_spmd

f32 = mybir.dt.float32
u32 = mybir.dt.uint32
AFT = mybir.ActivationFunctionType
Alu = mybir.AluOpType

N = 32768          # database points
M = 8192           # total queries per side
C = 256            # feature dim
NCORES = 8
MLOC = M // NCORES  # 1024 queries per core per side
P = 128            # partitions / m-tile size
NMT = MLOC // P    # 8 m-tiles
QTR = 8192         # resident rhs slice + DVE scan span
NQ = N // QTR      # 4 quarters
SPAN = 2048        # PSUM span (4 banks), double-buffered
SPQ = QTR // SPAN  # 4 spans per quarter
MM = 512           # matmul moving size (fp32 max; one PSUM bank)

# "gp_tt": ACT computes t1n=-(a2+b2) and drains PSUM; gpsimd TT adds them.
# "dve_stt": DVE scalar_tensor_tensor((negb2+nega2)+psum) does it all.
ADD_MODE = "gp_tt"
# in gp_tt mode, every DVE_EVERY-th span's add runs on DVE stt instead (0 = none)
DVE_EVERY = 0


def _split_waits_json(bir_bytes: bytes) -> bytes:
    import orjson

    d = orjson.loads(bir_bytes)
    ctr = [0]

    def mknop(engine, wait, debug):
        ctr[0] += 1
        return {
            "debug": debug,
            "engine": engine,
            "ins": [],
            "name": f"I-waitsplit-{ctr[0]}",
            "opcode": "NoOp",
            "outs": [],
            "sync_info": {"on_update": [], "on_wait": [wait]},
            "text_hint": "waitsplit",
        }

    for f in d.get("functions", []):
        for bb in f.get("blocks", []):
            insts = bb.get("instructions", [])
            out = []
            for i in insts:
                sy = i.get("sync_info")
                if sy:
                    waits = sy.get("on_wait") or []
                    keep = 0 if i.get("opcode") == "ISA" else 1
                    if len(waits) > keep:
                        for w in waits[: len(waits) - keep]:
                            out.append(mknop(i.get("engine"), w, i.get("debug", 0)))
                        sy["on_wait"] = waits[len(waits) - keep:]
                out.append(i)
            bb["instructions"] = out
    return orjson.dumps(d)


def _install_waitsplit():
    import concourse.bass_utils as bu
    import concourse.bass2jax as b2j

    if getattr(bu, "_waitsplit_installed", False):
        return
    orig = bu.compile_bir_kernel

    def patched(bir_json, tmpdir, neff_name="file.neff", **kw):
        return orig(_split_waits_json(bir_json), tmpdir, neff_name, **kw)

    bu.compile_bir_kernel = patched
    b2j.compile_bir_kernel = patched
    bu._waitsplit_installed = True


def _build():
    nc = bass.Bass()
    d = {}
    d["iota32"] = nc.dram_tensor("iota32", [1, 32], f32, kind="ExternalInput")
    d["offvec32"] = nc.dram_tensor("offvec32", [1, 32], f32, kind="ExternalInput")
    for s in ("s", "t"):
        d[f"feats_{s}"] = nc.dram_tensor(f"feats_{s}", [N, C], f32, kind="ExternalInput")
        # bT replicated at partition bases 0/32/64/96 (rows 32g..32g+2)
        d[f"bTrep_{s}"] = nc.dram_tensor(f"bTrep_{s}", [12, N], f32, kind="ExternalInput")
        d[f"negb2_{s}"] = nc.dram_tensor(f"negb2_{s}", [1, N], f32, kind="ExternalInput")
        # 2*aT replicated at partition bases 0/32/64/96
        d[f"aPack_{s}"] = nc.dram_tensor(f"aPack_{s}", [99, MLOC], f32, kind="ExternalInput")
        # [128, NMT]: partition p, col t = -a2[t*128 + p]
        d[f"na2_{s}"] = nc.dram_tensor(f"na2_{s}", [P, NMT], f32, kind="ExternalInput")
        d[f"out_{s}"] = nc.dram_tensor(f"out_{s}", [MLOC, C], f32, kind="ExternalOutput")

    span_ctr = [0]

    with TileContext(nc) as tc:
        with (
            tc.tile_pool(name="const", bufs=1) as cp,
            tc.tile_pool(name="rhs", bufs=1) as rp,
            tc.tile_pool(name="nb2", bufs=2) as np2,
            tc.tile_pool(name="score", bufs=2) as scp,
            tc.tile_pool(name="tmp", bufs=3) as tp,
            tc.tile_pool(name="cand", bufs=1) as cdp,
            tc.tile_pool(name="small", bufs=3) as sp,
            tc.tile_pool(name="gout", bufs=2) as gp,
            tc.tile_pool(name="ps", bufs=2, space="PSUM") as psp,
        ):
            iota32_t = cp.tile([P, 32], f32, tag="iota32")
            nc.sync.dma_start(iota32_t[:], d["iota32"][0:1, :].to_broadcast([P, 32]))
            off32_t = cp.tile([P, 32], f32, tag="off32")
            nc.sync.dma_start(off32_t[:], d["offvec32"][0:1, :].to_broadcast([P, 32]))

            aPack_tiles = {}
            na2_tiles = {}
            for s in ("s", "t"):
                aP = cp.tile([99, MLOC], f32, name=f"aPack_{s}", tag=f"aPack_{s}")
                nc.sync.dma_start(aP[:], d[f"aPack_{s}"][:])
                aPack_tiles[s] = aP
                na2_t = cp.tile([P, NMT], f32, name=f"na2_{s}", tag=f"na2_{s}")
                nc.sync.dma_start(na2_t[:], d[f"na2_{s}"][:])
                na2_tiles[s] = na2_t

            for s in ("s", "t"):
                aP = aPack_tiles[s]
                na2_t = na2_tiles[s]
                q8v_all = [cdp.tile([P, 32], f32, name=f"q8v{s}{t}", tag=f"q8v{s}{t}")
                           for t in range(NMT)]
                q8i_all = [cdp.tile([P, 32], u32, name=f"q8i{s}{t}", tag=f"q8i{s}{t}")
                           for t in range(NMT)]

                for Q in range(NQ):
                    bTrep_Q = rp.tile([99, QTR], f32, tag="bTrep_Q")
                    for g in range(4):
                        nc.sync.dma_start(
                            bTrep_Q[32 * g:32 * g + 3, :],
                            d[f"bTrep_{s}"][3 * g:3 * g + 3, Q * QTR:(Q + 1) * QTR])
                    nb2_Q = np2.tile([P, QTR], f32, tag="nb2_Q")
                    nc.sync.dma_start(
                        nb2_Q[:],
                        d[f"negb2_{s}"][0:1, Q * QTR:(Q + 1) * QTR].to_broadcast([P, QTR]))

                    for t in range(NMT):
                        sb = scp.tile([P, QTR], f32, tag="sb")
                        nega2 = na2_t[:, t:t + 1]
                        for q in range(SPQ):
                            pst = psp.tile([P, SPAN], f32, tag="ps")
                            for g in range(4):
                                nc.tensor.matmul(
                                    pst[:, g * MM:(g + 1) * MM],
                                    lhsT=aP[32 * g:32 * g + 3, t * P:(t + 1) * P],
                                    rhs=bTrep_Q[32 * g:32 * g + 3,
                                                q * SPAN + g * MM:q * SPAN + (g + 1) * MM],
                                    start=True, stop=True,
                                    tile_position=(32 * g, 0))
                            span_ctr[0] += 1
                            use_dve = (ADD_MODE == "dve_stt") or (
                                DVE_EVERY and (span_ctr[0] % DVE_EVERY == 0))
                            if use_dve:
                                nc.vector.scalar_tensor_tensor(
                                    out=sb[:, q * SPAN:(q + 1) * SPAN],
                                    in0=nb2_Q[:, q * SPAN:(q + 1) * SPAN],
                                    scalar=nega2,
                                    in1=pst[:],
                                    op0=Alu.add, op1=Alu.add)
                            else:
                                sbs = sb[:, q * SPAN:(q + 1) * SPAN]
                                nc.scalar.activation(
                                    sbs, nb2_Q[:, q * SPAN:(q + 1) * SPAN],
                                    AFT.Identity, bias=nega2, scale=1.0)
                                tmp = tp.tile([P, SPAN], f32, tag="tmp")
                                nc.scalar.copy(tmp[:], pst[:])
                                nc.gpsimd.tensor_tensor(
                                    sbs, sbs, tmp[:], op=Alu.add)
                        qv = q8v_all[t][:, 8 * Q:8 * Q + 8]
                        nc.vector.max(out=qv, in_=sb[:])
                        nc.vector.max_index(
                            out=q8i_all[t][:, 8 * Q:8 * Q + 8], in_max=qv, in_values=sb[:])

                for t in range(NMT):
                    candi_f = sp.tile([P, 32], f32, tag="candi_f")
                    nc.vector.tensor_copy(candi_f[:], q8i_all[t][:])
                    candi_g = sp.tile([P, 32], f32, tag="candi_g")
                    nc.vector.tensor_tensor(candi_g[:], candi_f[:], off32_t[:], op=Alu.add)
                    g8v = sp.tile([P, 8], f32, tag="g8v")
                    nc.vector.max(out=g8v[:], in_=q8v_all[t][:])
                    g8i = sp.tile([P, 8], u32, tag="g8i")
                    nc.vector.max_index(out=g8i[:], in_max=g8v[:], in_values=q8v_all[t][:])
                    slot1f = sp.tile([P, 1], f32, tag="slot1f")
                    nc.vector.tensor_copy(slot1f[:], g8i[:, 1:2])
                    msk = sp.tile([P, 32], f32, tag="msk")
                    nc.vector.tensor_tensor(
                        msk[:], iota32_t[:], slot1f[:, 0:1].to_broadcast([P, 32]),
                        op=Alu.is_equal)
                    prod = sp.tile([P, 32], f32, tag="prod")
                    nc.vector.tensor_tensor(prod[:], msk[:], candi_g[:], op=Alu.mult)
                    sec = sp.tile([P, 1], f32, tag="sec")
                    nc.vector.reduce_sum(sec[:], prod[:], axis=mybir.AxisListType.X)
                    sec_u = sp.tile([P, 1], u32, tag="sec_u")
                    nc.vector.tensor_copy(sec_u[:], sec[:])
                    g = gp.tile([P, C], f32, tag="g")
                    nc.gpsimd.indirect_dma_start(
                        out=g[:],
                        out_offset=None,
                        in_=d[f"feats_{s}"][:],
                        in_offset=bass.IndirectOffsetOnAxis(ap=sec_u[:, :1], axis=0),
                    )
                    nc.sync.dma_start(d[f"out_{s}"][t * P:(t + 1) * P, :], g[:])
    return nc


_NC_CACHE = {}


def _get_nc():
    if "nc" not in _NC_CACHE:
        _install_waitsplit()
        _NC_CACHE["nc"] = _build()
    return _NC_CACHE["nc"]


def kernel(src, tgt, src_coords, tgt_coords, src_shortcut_coords, tgt_shortcut_coords):
    src = np.ascontiguousarray(np.asarray(src, np.float32))
    tgt = np.ascontiguousarray(np.asarray(tgt, np.float32))
    bs = np.asarray(src_coords, np.float32)
    bt = np.asarray(tgt_coords, np.float32)
    a_s = np.asarray(src_shortcut_coords, np.float32)
    a_t = np.asarray(tgt_shortcut_coords, np.float32)

    nc = _get_nc()

    def side_inputs(tag, feats, bcoord, acoord):
        b2 = (bcoord * bcoord).sum(1, dtype=np.float32).astype(np.float32)
        a2 = (acoord * acoord).sum(1, dtype=np.float32).astype(np.float32)
        bT = np.ascontiguousarray(bcoord.T)                  # [3, N]
        bTrep = np.empty((12, N), np.float32)
        for g in range(4):
            bTrep[3 * g:3 * g + 3] = bT
        d = {
            f"feats_{tag}": feats,
            f"bTrep_{tag}": bTrep,
            f"negb2_{tag}": np.ascontiguousarray(-b2[None, :]),
        }
        per_core = []
        for c in range(NCORES):
            sl = slice(c * MLOC, (c + 1) * MLOC)
            aT = (2.0 * acoord[sl]).T.astype(np.float32)     # [3, MLOC]
            aPack = np.zeros((99, MLOC), np.float32)
            for g in range(4):
                aPack[32 * g:32 * g + 3] = aT
            na2 = (-a2[sl]).reshape(NMT, P).T                # [128, NMT]
            per_core.append({
                f"aPack_{tag}": aPack,
                f"na2_{tag}": np.ascontiguousarray(na2),
            })
        return d, per_core

    shared_s, core_s = side_inputs("s", src, bs, a_s)
    shared_t, core_t = side_inputs("t", tgt, bt, a_t)

    iota32 = np.arange(32, dtype=np.float32)[None, :]
    offvec32 = (np.repeat(np.arange(4) * QTR, 8).astype(np.float32))[None, :]
    in_maps = []
    for c in range(NCORES):
        m = {"iota32": iota32, "offvec32": offvec32}
        m.update(shared_s)
        m.update(shared_t)
        m.update(core_s[c])
        m.update(core_t[c])
        in_maps.append(m)

    import os
    import time as _time
    trace = bool(os.environ.get("KERNEL_TRACE"))
    last_err = None
    for _attempt in range(3):
        try:
            r = run_bass_kernel_spmd(
                nc, in_maps, core_ids=list(range(NCORES)), trace=trace)
            break
        except Exception as e:  # transient NRT_EXEC_UNIT_UNRECOVERABLE etc.
            last_err = e
            _time.sleep(3.0)
    else:
        raise last_err
    LAST_RESULTS["r"] = r
    res = r.results
    out_src = np.concatenate([res[c]["out_s"] for c in range(NCORES)], axis=0)
    out_tgt = np.concatenate([res[c]["out_t"] for c in range(NCORES)], axis=0)
    return (out_src, out_tgt)


LAST_RESULTS = {}
